# revision 1
# baseline (speedup 1.0000x reference)
"""Trainium2 Bass kernel for nn_Decoder_35837207118002 (retrieval_knn).

Problem: b=1, n_pre=8192, n_cur=16384, K=8.
  cur2pre[j] = argmin_i D[i,j]           (nearest pre for each cur)
  knn_idx[i] = 8 smallest D[i,:] (indices into cur)
  mask[i,k]  = (cur2pre[knn_idx[i,k]] == i)
  out[i]     = sum_k mask*dist / upsample[i],  dist = ||pre_i - cur_knn||

Sharding: over pre across 8 cores (1024 pre points per core), cur
replicated.  Each core computes the negated distance matrix
ND = 2*dot - psq - csq = -D against all 16384 cur points via K=5
augmented PE matmuls (host precomputes the squared norms), finds the
row top-8 per cur-quarter with the DVE max/max_index top-8 primitive
(32 candidates per pre row, a superset of the global top-8), and
reduces the column max (== -min_i D over the core's slice) via GPSIMD
running-max + PE-transpose partition reduction.

The device returns candidate values/indices and the per-core column
max; the host combines the 8 column-max slices, applies the
argmin-mask (bitwise value equality), membership threshold, exact
distance, and upsample division -- a trivially small reduction
(~256K elements of numpy) compared to the 134M-element matrix work.
"""

import numpy as np

import concourse.bass as bass
import concourse.tile as tile
import concourse.mybir as mybir
import concourse.bass_utils as bass_utils
from concourse.masks import make_identity

F32 = mybir.dt.float32
U16 = mybir.dt.uint16
AX = mybir.AxisListType
OP = mybir.AluOpType

N_CORES = 8
P = 128
N_PRE = 8192
N_CUR = 16384
K = 8
PRE_CORE = N_PRE // N_CORES      # 1024
NCH = PRE_CORE // P              # 8 pre chunks per core
NQ = 4                           # cur quarters
QW = N_CUR // NQ                 # 4096
CPR = NQ * K                     # 32 candidates per pre row
NCAND = NCH * CPR                # 256 candidate slots per partition

# This walrus build rejects 2-input TensorTensor on the Pool (gpsimd)
# engine.  The column-max merge options:
#   "dma": accumulating SBUF->SBUF DMA (accum_op=max) on the idle DMA
#          engines (SWDGE via gpsimd queue).
#   "dve": scalar_tensor_tensor on DVE reading PSUM directly.
MERGE = "dve"
# fp32r would stream matmuls at 1 cycle/row (vs 4 for plain fp32) but it
# is a reduced-precision format (inputs must be pre-rounded to fp32r) --
# the knn selection needs exact fp32, so this stays off.
F32R_MM = False

_COMPILED = {}


def _split_excess_drain_waits(nc, limit=1):
    """This walrus build encodes very few sem-waits per instruction (a
    Drain tops out at ONE).  Hoist excess waits onto preceding
    single-wait NoOps on the same engine (a NoOp doesn't stall the
    engine pipeline the way a Drain would)."""
    for f in nc.m.functions:
        for bb in f.blocks:
            insts = list(bb.instructions)
            out = []
            changed = False
            for inst in insts:
                si = inst.sync_info
                waits = list(si.on_wait) if si and si.on_wait else []
                if len(waits) > limit:
                    for kk, w in enumerate(waits[:-limit]):
                        out.append(
                            mybir.InstNoOp(
                                name=f"{inst.name}-wsplit{kk}",
                                engine=inst.engine,
                                ins=[],
                                outs=[],
                                sync_info=mybir.SyncInfo(on_wait=[w], on_update=[]),
                            )
                        )
                    si.on_wait = waits[-limit:]
                    inst.sync_info = si
                    changed = True
                out.append(inst)
            if changed:
                bb.instructions = out


def build_kernel():
    nc = bass.Bass("TRN2", target_bir_lowering=False, debug=False,
                   num_devices=N_CORES)

    pre_aug = nc.dram_tensor("pre_aug", [5, PRE_CORE], F32, kind="ExternalInput").ap()
    cur_aug = nc.dram_tensor("cur_aug", [5, N_CUR], F32, kind="ExternalInput").ap()
    oV = nc.dram_tensor("oV", [P, NCAND], F32, kind="ExternalOutput").ap()
    oI = nc.dram_tensor("oI", [P, NCAND], U16, kind="ExternalOutput").ap()
    oM = nc.dram_tensor("oM", [P, P], F32, kind="ExternalOutput").ap()

    with tile.TileContext(nc) as tc:
        with (
            tc.tile_pool(name="const", bufs=1) as const_pool,
            tc.tile_pool(name="s", bufs=3) as s_pool,
            tc.tile_pool(name="m", bufs=2) as m_pool,
            tc.tile_pool(name="mmps", bufs=3, space="PSUM") as mm_psum,
            tc.tile_pool(name="trps", bufs=2, space="PSUM") as tr_psum,
        ):
            cur_aug_sb = const_pool.tile([5, N_CUR], F32)
            nc.sync.dma_start(cur_aug_sb[:], cur_aug[:])
            pre_aug_sb = const_pool.tile([5, PRE_CORE], F32)
            nc.sync.dma_start(pre_aug_sb[:], pre_aug[:])
            ident = const_pool.tile([P, P], F32)
            make_identity(nc, ident[:])

            V_all = const_pool.tile([P, NCAND], F32)
            I_all = const_pool.tile([P, NCAND], U16)
            M_compact = const_pool.tile([P, P], F32)

            for q in range(NQ):
                Mrun = m_pool.tile([P, QW], F32, name=f"Mrun_q{q}", tag="Mrun")
                for pc in range(NCH):
                    S = s_pool.tile([P, QW], F32, name=f"S_q{q}_p{pc}", tag="S")
                    lhsT = pre_aug_sb[:, pc * P:(pc + 1) * P]
                    if F32R_MM:
                        lhsT = lhsT.bitcast(mybir.dt.float32r)
                    for t in range(QW // 1024):
                        ps = mm_psum.tile([P, 1024], F32, name=f"ps_{q}_{pc}_{t}",
                                          tag="mmps")
                        for u in range(2):
                            col = q * QW + t * 1024 + u * 512
                            rhs = cur_aug_sb[:, col:col + 512]
                            if F32R_MM:
                                rhs = rhs.bitcast(mybir.dt.float32r)
                            nc.tensor.matmul(
                                ps[:, u * 512:(u + 1) * 512],
                                lhsT,
                                rhs,
                                start=True, stop=True,
                            )
                        nc.scalar.copy(S[:, t * 1024:(t + 1) * 1024], ps[:])
                    off = pc * CPR + q * K
                    nc.vector.max(out=V_all[:, off:off + K], in_=S[:])
                    nc.vector.max_index(I_all[:, off:off + K],
                                        V_all[:, off:off + K], S[:])
                    # column-max merge: ACT copy for the first chunk, DVE
                    # elementwise max for the rest (one op per chunk).
                    if pc == 0:
                        nc.scalar.copy(Mrun[:], S[:])
                    else:
                        nc.vector.tensor_max(Mrun[:], Mrun[:], S[:])
                # partition-direction reduction of Mrun via PE transposes,
                # packed 4 per PSUM bank pair and reduced in groups.
                for tg in range(QW // P // 4):  # 8 groups of 4 tiles
                    trp = tr_psum.tile([P, 4, P], F32, name=f"trp_{q}_{tg}",
                                       tag="trps")
                    for t4 in range(4):
                        t = tg * 4 + t4
                        nc.tensor.transpose(
                            trp[:, t4], Mrun[:, t * P:(t + 1) * P], ident[:])
                    base = q * (QW // P) + tg * 4
                    nc.vector.tensor_reduce(
                        M_compact[:, base:base + 4], trp[:],
                        axis=AX.X, op=OP.max,
                    )

            nc.sync.dma_start(oV[:], V_all[:])
            nc.sync.dma_start(oI[:], I_all[:])
            nc.sync.dma_start(oM[:], M_compact[:])

    _split_excess_drain_waits(nc)
    return nc


def _prep_inputs(pre_xyzs, cur_xyzs, upsample_num):
    """Host-side per-core input prep.  Stepwise fp32 norms to mirror the
    reference's _sqdist."""
    p = np.ascontiguousarray(pre_xyzs[0], dtype=np.float32)   # (3, 8192)
    c = np.ascontiguousarray(cur_xyzs[0], dtype=np.float32)   # (3, 16384)

    psq = ((p[0] * p[0] + p[1] * p[1]) + p[2] * p[2]).astype(np.float32)
    csq = ((c[0] * c[0] + c[1] * c[1]) + c[2] * c[2]).astype(np.float32)

    cur_aug = np.empty((5, N_CUR), np.float32)
    cur_aug[0:3] = 2.0 * c
    cur_aug[3] = -1.0
    cur_aug[4] = -csq

    in_maps = []
    for core in range(N_CORES):
        s = slice(core * PRE_CORE, (core + 1) * PRE_CORE)
        pre_aug = np.empty((5, PRE_CORE), np.float32)
        pre_aug[0:3] = p[:, s]
        pre_aug[3] = psq[s]
        pre_aug[4] = 1.0
        in_maps.append({"pre_aug": pre_aug, "cur_aug": cur_aug})
    return in_maps


def kernel(pre_xyzs, cur_xyzs, upsample_num, _run_kwargs=None):
    # The bass->PJRT path needs the axon (NeuronCore) jax backend; guard
    # against a host process that pinned jax to CPU for its reference.
    try:
        import jax
        if not any("NC" in str(d) for d in jax.devices()):
            jax.config.update("jax_platforms", "axon")
    except Exception:
        pass
    if "nc" not in _COMPILED:
        _COMPILED["nc"] = build_kernel()
    nc = _COMPILED["nc"]
    in_maps = _prep_inputs(pre_xyzs, cur_xyzs, upsample_num)
    try:
        res = bass_utils.run_bass_kernel_spmd(
            nc, in_maps, core_ids=list(range(N_CORES)), **(_run_kwargs or {}))
    except Exception:
        # One retry: the axon-tunneled devices occasionally come up wedged
        # (NRT_EXEC_UNIT_UNRECOVERABLE) and recover on the next execution.
        import time
        time.sleep(5)
        res = bass_utils.run_bass_kernel_spmd(
            nc, in_maps, core_ids=list(range(N_CORES)), **(_run_kwargs or {}))
    _COMPILED["last_results"] = res

    # ---- host-side masked reduction (tiny: ~256K elements) ----
    p = np.ascontiguousarray(pre_xyzs[0], dtype=np.float32)
    c = np.ascontiguousarray(cur_xyzs[0], dtype=np.float32)
    cur_pts = np.ascontiguousarray(c.T)                       # (16384, 3)
    up = np.ascontiguousarray(upsample_num[0], dtype=np.float32)

    m_global = np.max(
        [res.results[core]["oM"].T.reshape(-1) for core in range(N_CORES)],
        axis=0)                                               # (16384,)

    qoff = (np.arange(NQ, dtype=np.int32) * QW)[None, None, :, None]
    out = np.empty((1, N_PRE), np.float32)
    for core in range(N_CORES):
        V = res.results[core]["oV"]                           # (128, 256) f32
        I = res.results[core]["oI"].astype(np.int32)          # (128, 256)
        Ig = (I.reshape(P, NCH, NQ, K) + qoff).reshape(P, NCAND)
        Mg = m_global[Ig]
        mask = (V == Mg)
        # membership: >= 8th largest of the row's 32 candidates
        Vc = V.reshape(P, NCH, CPR)
        t8 = -np.partition(-Vc, K - 1, axis=2)[:, :, K - 1:K]
        memb = Vc >= t8
        # exact distance (reference formula) from gathered coordinates
        s = slice(core * PRE_CORE, (core + 1) * PRE_CORE)
        pre_b = np.ascontiguousarray(
            p[:, s].reshape(3, NCH, P).transpose(2, 1, 0))    # (128, 8, 3)
        CP = cur_pts[Ig].reshape(P, NCH, CPR, 3)
        diff = (CP - pre_b[:, :, None, :]).astype(np.float32)
        d2 = ((diff[..., 0] * diff[..., 0] + diff[..., 1] * diff[..., 1])
              + diff[..., 2] * diff[..., 2]).astype(np.float32)
        dist = np.sqrt(d2)
        contrib = (dist * (mask.reshape(P, NCH, CPR) & memb)).sum(
            -1, dtype=np.float32).astype(np.float32)          # (128, 8)
        ur = up[s].reshape(NCH, P)                            # (pc, p)
        out[0, s] = (contrib.T / ur).reshape(-1).astype(np.float32)
    return out



# revision 6
# speedup vs baseline: 2.1497x; 2.1497x over previous
"""Trainium2 Bass kernel for nn_Decoder_35837207118002 (retrieval_knn).

Problem: b=1, n_pre=8192, n_cur=16384, K=8.
  cur2pre[j] = argmin_i D[i,j]           (nearest pre for each cur)
  knn_idx[i] = 8 smallest D[i,:] (indices into cur)
  mask[i,k]  = (cur2pre[knn_idx[i,k]] == i)
  out[i]     = sum_k mask*dist / upsample[i],  dist = ||pre_i - cur_knn||

Sharding: pre split across 8 cores (1024 rows each), cur replicated.

Device strategy (per core), built around two empirical facts probed on
this stack: (1) the PE accumulates matmul products sequentially in
contraction-row order in fp32, bit-replicably from numpy; (2) PE cost is
independent of the contraction dim K (<=128) at 1 cycle/row for bf16.

  ND = 2*p.c - |p|^2 - |c|^2  (= -D) is computed as a K=24 bf16 matmul:
  every fp32 input is split into three exact bf16 pieces (h+m+l), and
  the 24 rank-1 terms reproduce full fp32-level precision at 4x the
  fp32 matmul rate.  Each [128, 2048] PSUM tile (two pre-chunks wide) is
  consumed twice, concurrently:
    - DVE tensor_reduce (windowed max, 32 cur cols/window) -> W (bf16),
      the per-row window maxima used for top-8 *selection* only;
    - ACT drains it to SBUF, and once a full cur-chunk x 8 pre-chunks
      block [128, 8, 1024] is resident, the gpsimd engine's C-axis
      tensor_reduce computes the exact fp32 column max over all 1024
      pre rows in one instruction -> M.
  All four engines run concurrently; the DVE windowed pass (~137us) is
  the critical path.  No max_index / no cross-tile merge is needed.

Host side: selects >=top-8 windows per row from W (bf16 ranking with
ties included -- provably covers the true top-8), recomputes candidate
ND values in reference-style fp32 for the knn membership, replicates
the device's sequential 24-term sum bitwise for the final 8 pairs/row,
and computes the argmin mask as a bitwise equality against the device
column max M.  The remaining reduction (sqrt / mask / upsample) follows
the reference formulas exactly.
"""

import numpy as np
import ml_dtypes

import concourse.bass as bass
import concourse.tile as tile
import concourse.mybir as mybir
import concourse.bass_utils as bass_utils

F32 = mybir.dt.float32
BF16 = mybir.dt.bfloat16
AX = mybir.AxisListType
OP = mybir.AluOpType

N_CORES = 8
P = 128
N_PRE = 8192
N_CUR = 16384
K = 8
PRE_CORE = N_PRE // N_CORES      # 1024
NCH = PRE_CORE // P              # 8 pre chunks of 128 rows
KB = 24                          # bf16^3 augmented contraction rows
WIN = 32                         # cur columns per selection window
NWIN = N_CUR // WIN              # 512 windows per row
CHUNK = 1024                     # cur columns per column-max block
NCHUNK = N_CUR // CHUNK          # 16
SEL_CAP = 16                     # max windows/row the host will expand

_COMPILED = {}


def _split_excess_drain_waits(nc, limit=1):
    """This walrus build encodes very few sem-waits per instruction (a
    Drain tops out at ONE).  Hoist excess waits onto preceding
    single-wait NoOps on the same engine."""
    for f in nc.m.functions:
        for bb in f.blocks:
            insts = list(bb.instructions)
            out = []
            changed = False
            for inst in insts:
                si = inst.sync_info
                waits = list(si.on_wait) if si and si.on_wait else []
                if len(waits) > limit:
                    for kk, w in enumerate(waits[:-limit]):
                        out.append(
                            mybir.InstNoOp(
                                name=f"{inst.name}-wsplit{kk}",
                                engine=inst.engine,
                                ins=[],
                                outs=[],
                                sync_info=mybir.SyncInfo(on_wait=[w], on_update=[]),
                            )
                        )
                    si.on_wait = waits[-limit:]
                    inst.sync_info = si
                    changed = True
                out.append(inst)
            if changed:
                bb.instructions = out


def build_kernel():
    nc = bass.Bass("TRN2", target_bir_lowering=False, debug=False,
                   num_devices=N_CORES)

    pre_b = nc.dram_tensor("pre_b", [KB, PRE_CORE], BF16, kind="ExternalInput").ap()
    cur_b = nc.dram_tensor("cur_b", [KB, N_CUR], BF16, kind="ExternalInput").ap()
    oW = nc.dram_tensor("oW", [P, NCH * NWIN], BF16, kind="ExternalOutput").ap()
    # per-chunk, per-pre-chunk partition maxima: 16 chunks x 8192 values,
    # shipped as [128, 64] blocks (the host finishes the 8-way pc max)
    oM = nc.dram_tensor("oM", [P, NCHUNK * NCH * CHUNK // P], F32,
                        kind="ExternalOutput").ap()

    with tile.TileContext(nc) as tc:
        with (
            tc.tile_pool(name="const", bufs=1) as const_pool,
            tc.tile_pool(name="sbig", bufs=2) as sbig_pool,
            tc.tile_pool(name="mrow", bufs=2) as mrow_pool,
            tc.tile_pool(name="mmps", bufs=2, space="PSUM") as mm_psum,
        ):
            cur_sb = const_pool.tile([KB, N_CUR], BF16)
            nc.sync.dma_start(cur_sb[:], cur_b[:])
            pre_sb = const_pool.tile([KB, PRE_CORE], BF16)
            nc.sync.dma_start(pre_sb[:], pre_b[:])

            W_all = const_pool.tile([P, NCH, NWIN], BF16)

            for ch in range(NCHUNK):
                col0 = ch * CHUNK
                rhs = cur_sb[:, col0:col0 + CHUNK]
                S_big = sbig_pool.tile([P, NCH, CHUNK], F32,
                                       name=f"S_{ch}", tag="sbig")
                for q in range(NCH // 2):  # pre-chunk pairs
                    pt = mm_psum.tile([P, 2 * CHUNK], F32,
                                      name=f"pt_{ch}_{q}", tag="mm")
                    for e in range(2):
                        pc = 2 * q + e
                        lhsT = pre_sb[:, pc * P:(pc + 1) * P]
                        for u in range(CHUNK // 512):
                            nc.tensor.matmul(
                                pt[:, e * CHUNK + u * 512:
                                   e * CHUNK + (u + 1) * 512],
                                lhsT, rhs[:, u * 512:(u + 1) * 512],
                                start=True, stop=True,
                            )
                    # per-row window maxima (selection only -> bf16 out)
                    nc.vector.tensor_reduce(
                        W_all[:, 2 * q:2 * q + 2,
                              ch * (CHUNK // WIN):(ch + 1) * (CHUNK // WIN)],
                        pt[:].rearrange("p (w c) -> p w c", c=WIN),
                        axis=AX.X, op=OP.max,
                    )
                    # drain to SBUF for the column-max pass
                    nc.scalar.copy(
                        S_big[:, 2 * q:2 * q + 2, :].rearrange("p a b -> p (a b)"),
                        pt[:])
                # exact fp32 column max over all 1024 pre rows of the core
                Mrow = mrow_pool.tile([1, NCH * CHUNK], F32,
                                      name=f"M_{ch}", tag="mrow")
                nc.gpsimd.tensor_reduce(
                    Mrow[:], S_big[:], axis=AX.C, op=OP.max)
                nc.sync.dma_start(
                    oM[:, ch * (NCH * CHUNK // P):(ch + 1) * (NCH * CHUNK // P)],
                    Mrow[:])

            nc.sync.dma_start(oW[:], W_all[:].rearrange("p a b -> p (a b)"))

    _split_excess_drain_waits(nc)
    return nc


def _split3(x):
    """fp32 -> three exact bf16 pieces (h, m, l), x == h + m + l + O(2^-24)."""
    x = np.asarray(x, np.float32)
    h = x.astype(ml_dtypes.bfloat16).astype(np.float32)
    r = (x - h).astype(np.float32)
    m = r.astype(ml_dtypes.bfloat16).astype(np.float32)
    l = (r - m).astype(np.float32).astype(ml_dtypes.bfloat16).astype(np.float32)
    return h, m, l


def _build_aug(p, c):
    """24-row bf16 augmented matrices with ND = sum_k A[k,i]*B[k,j].

    Row order puts the main (hh) terms first so the sequential PE
    accumulation follows the reference's 5-term magnitude profile.
    """
    psq = ((p[0] * p[0] + p[1] * p[1]) + p[2] * p[2]).astype(np.float32)
    csq = ((c[0] * c[0] + c[1] * c[1]) + c[2] * c[2]).astype(np.float32)
    ph, pm, pl = _split3(p)
    ch, cm, cl = _split3(2.0 * c)
    qh, qm, ql = _split3(psq)
    sh, sm, sl = _split3(csq)
    A = np.zeros((KB, p.shape[1]), np.float32)
    B = np.zeros((KB, c.shape[1]), np.float32)
    r = 0
    A[r:r+3] = ph; B[r:r+3] = ch; r += 3
    A[r] = qh; B[r] = -1.0; r += 1
    A[r] = 1.0; B[r] = -sh; r += 1
    A[r:r+3] = ph; B[r:r+3] = cm; r += 3
    A[r:r+3] = pm; B[r:r+3] = ch; r += 3
    A[r] = qm; B[r] = -1.0; r += 1
    A[r] = 1.0; B[r] = -sm; r += 1
    A[r:r+3] = ph; B[r:r+3] = cl; r += 3
    A[r:r+3] = pl; B[r:r+3] = ch; r += 3
    A[r:r+3] = pm; B[r:r+3] = cm; r += 3
    A[r] = ql; B[r] = -1.0; r += 1
    A[r] = 1.0; B[r] = -sl; r += 1
    assert r == KB
    return A.astype(ml_dtypes.bfloat16), B.astype(ml_dtypes.bfloat16)


def _nd_device(A, B, ii, jj):
    """Bit-exact replica of the device 24-term sequential fp32 sum for
    index arrays ii (pre), jj (cur)."""
    acc = np.zeros(ii.shape, np.float32)
    for k in range(KB):
        t = (A[k].astype(np.float32)[ii]
             * B[k].astype(np.float32)[jj]).astype(np.float32)
        acc = (acc + t).astype(np.float32)
    return acc


def kernel(pre_xyzs, cur_xyzs, upsample_num, _run_kwargs=None):
    try:
        import jax
        if not any("NC" in str(d) for d in jax.devices()):
            jax.config.update("jax_platforms", "axon")
    except Exception:
        pass
    if "nc" not in _COMPILED:
        _COMPILED["nc"] = build_kernel()
    nc = _COMPILED["nc"]

    p = np.ascontiguousarray(pre_xyzs[0], dtype=np.float32)   # (3, 8192)
    c = np.ascontiguousarray(cur_xyzs[0], dtype=np.float32)   # (3, 16384)
    up = np.ascontiguousarray(upsample_num[0], dtype=np.float32)

    A, B = _build_aug(p, c)
    in_maps = []
    for core in range(N_CORES):
        s = slice(core * PRE_CORE, (core + 1) * PRE_CORE)
        in_maps.append({"pre_b": np.ascontiguousarray(A[:, s]),
                        "cur_b": np.ascontiguousarray(B)})

    try:
        res = bass_utils.run_bass_kernel_spmd(
            nc, in_maps, core_ids=list(range(N_CORES)), **(_run_kwargs or {}))
    except Exception:
        import time
        time.sleep(5)
        res = bass_utils.run_bass_kernel_spmd(
            nc, in_maps, core_ids=list(range(N_CORES)), **(_run_kwargs or {}))
    _COMPILED["last_results"] = res

    # ---- host reduction ----
    # column max over all pre (exact fp32 device values): finish the
    # 8-way pre-chunk max and the 8-way core max on host (bitwise max of
    # device-computed values).
    m_cores = []
    for core in range(N_CORES):
        Md = res.results[core]["oM"]                   # (128, 1024)
        # chunk ch block [:, ch*64:(ch+1)*64] holds Mrow flat: element
        # (p, ch*64+q) = Mrow[p*64+q], flat index f = pc*1024 + col
        m = (Md.reshape(P, NCHUNK, NCH * CHUNK // P)
             .transpose(1, 0, 2).reshape(NCHUNK, NCH, CHUNK)
             .max(axis=1).reshape(-1))                 # (16384,)
        m_cores.append(m)
    m_global = np.max(m_cores, axis=0)                 # (16384,) fp32

    # reference-style fp32 ingredients for membership selection
    psq = ((p[0] * p[0] + p[1] * p[1]) + p[2] * p[2]).astype(np.float32)
    csq = ((c[0] * c[0] + c[1] * c[1]) + c[2] * c[2]).astype(np.float32)
    cur_pts = np.ascontiguousarray(c.T)                # (16384, 3)

    out = np.empty((1, N_PRE), np.float32)
    for core in range(N_CORES):
        Wd = res.results[core]["oW"].reshape(P, NCH, NWIN)  # bf16
        # rows: global pre index = core*1024 + pc*128 + partition
        Wv = (np.ascontiguousarray(Wd.transpose(1, 0, 2))
              .reshape(PRE_CORE, NWIN))               # [row_in_core, win] bf16
        Wf = Wv.astype(np.float32)
        # ties-included >= 8th-largest window selection, capped at SEL_CAP
        t8 = -np.partition(-Wf, K - 1, axis=1)[:, K - 1:K]
        selmask = Wf >= t8
        # rank by value to cap selection deterministically
        order = np.argsort(-Wf, axis=1, kind="stable")[:, :SEL_CAP]
        sel = np.where(
            np.take_along_axis(selmask, order, axis=1), order, -1)  # (1024, 16)

        rows = np.arange(PRE_CORE)
        gi = core * PRE_CORE + rows                    # global pre indices
        # candidate cur columns: selected windows expanded to 32 cols
        wsel = sel[:, :, None] * WIN + np.arange(WIN)[None, None, :]
        wsel = wsel.reshape(PRE_CORE, -1)              # (1024, 512)
        valid = sel[:, :, None].repeat(WIN, 2).reshape(PRE_CORE, -1) >= 0

        # reference-style candidate values for knn membership
        ii = np.repeat(gi, wsel.shape[1]).reshape(PRE_CORE, -1)
        jj = np.where(valid, wsel, 0)
        dotpc = (p[0][ii] * c[0][jj] + p[1][ii] * c[1][jj]
                 + p[2][ii] * c[2][jj]).astype(np.float32)
        dref = ((psq[ii] + csq[jj]) - 2.0 * dotpc).astype(np.float32)
        dref[~valid] = np.inf
        # top-8 smallest with index tie-break (reference top_k semantics)
        ordc = np.lexsort((jj, dref), axis=1)[:, :K]   # (1024, 8)
        j8 = np.take_along_axis(jj, ordc, axis=1)      # chosen cur indices

        # device-replicated values for the argmin mask
        i8 = np.repeat(gi, K).reshape(PRE_CORE, K)
        nd_dev = _nd_device(A, B, i8, j8)
        mask = (nd_dev == m_global[j8])

        # exact distances (reference formula)
        pre_pts = p.T[gi]                              # (1024, 3)
        diff = (cur_pts[j8] - pre_pts[:, None, :]).astype(np.float32)
        d2 = ((diff[..., 0] * diff[..., 0] + diff[..., 1] * diff[..., 1])
              + diff[..., 2] * diff[..., 2]).astype(np.float32)
        dist = np.sqrt(d2)
        contrib = (dist * mask).sum(-1, dtype=np.float32).astype(np.float32)
        s = slice(core * PRE_CORE, (core + 1) * PRE_CORE)
        out[0, s] = contrib / up[s]
    return out
